# revision 19
# baseline (speedup 1.0000x reference)
"""BlockSparseMLA Trainium2 kernel (v2).

Sharding: 8 cores = 2 batches x 4 seq-quarters. Each core computes all 16
heads for its 512 queries. Host does block scoring / top-k, gathers
selected positions, builds the causal mask over selected keys, and patches
degenerate all-masked rows with a host-computed rank-1 fallback.

Device layouts are all "transposed" (feature dim on partitions):
  qT [c=16h*64, s]  kT [c, keys]  v [keys, c]  scoresT/expT [keys, s]
  yT [c, s]  out [s, dout]
All matmul operands are bf16 (fp32r measured ~1.8 cyc/row vs bf16 1.0);
PSUM accumulates fp32.

v2 structure:
- ~32 warmup matmuls on the perm tile while DMA streams in, so the PE HAM
  clock-gate reaches 8/8 before real work starts and stays there.
- Stages: A latent -> B kT+RoPE -> C v -> fused {q-proj chunk p, RoPE,
  scores, exp, mask, Z, PV, 1/Z, yT} per head-pair p -> F out-proj.
  Fusing D+E gives the PE queue q-proj work of pair p+1 to chew on while
  the scalar/vector/gpsimd engines run pair p's softmax chain.
- Scores use 64-partition row tiles (hi=0/1 in different row groups, can
  overlap); Z/PV are 64-column col tiles (hi halves of the PSUM tile) with
  an unpadded v layout.
- 1/Z via reciprocal_approx_fast (z=0 rows give garbage confined to dead
  queries, overwritten on host).
- Stage F runs st-outer so each 128-query output strip is evacuated and
  DMA'd while the next accumulates; out is bf16, host casts to fp32.
"""

import sys

import numpy as np

sys.path.insert(0, "/opt/trn_rl_repo")

from contextlib import ExitStack

import ml_dtypes

import concourse.bacc as bacc
import concourse.bass as bass
import concourse.mybir as mybir
import concourse.tile as tile

B, S, D = 2, 2048, 1024
H, HD, R = 16, 64, 128
BLOCK, TOPK = 64, 4
ROPE_BASE = 100000.0
SQ = S // 4
KEYS = TOPK * BLOCK  # 256
CK = D // 128  # c chunks (2 heads each)
DK = D // 128  # d chunks
F32 = mybir.dt.float32
BF16 = mybir.dt.bfloat16
NPBF16 = ml_dtypes.bfloat16
N_WARMUP = 48  # ~5us of N=128 matmuls at the cold 1.2 GHz clock


def _bf16(a):
    return np.ascontiguousarray(np.asarray(a, dtype=np.float32).astype(NPBF16))


def _perm():
    """[128, 128] block-diag rotate-half permutation: out[p] = in[p^32
    within each 64-block] (symmetric)."""
    P = np.zeros((128, 128), np.float32)
    for pp in range(128):
        blk, e = divmod(pp, 64)
        s = blk * 64 + (e + 32 if e < 32 else e - 32)
        P[s, pp] = 1.0
    return P


def host_prep(x, w_q, w_kv_down, w_kv_up, w_out, w_scorer):
    """Returns (in_maps for 8 cores, qmin[B], fb_rows[B, D])."""
    x = np.asarray(x, dtype=np.float32)
    nb = S // BLOCK

    reps = x.reshape(B, nb, BLOCK, D).mean(axis=2)
    scores = reps @ np.asarray(w_scorer, np.float32)[0]
    top = np.argsort(-scores, axis=1, kind="stable")[:, :TOPK]
    sel_blocks = np.sort(top, axis=1)
    qmin = sel_blocks[:, 0] * BLOCK
    sel_pos = (
        sel_blocks[:, :, None] * BLOCK + np.arange(BLOCK)[None, None, :]
    ).reshape(B, KEYS)

    half = np.arange(0, HD, 2, dtype=np.float32) / np.float32(HD)
    inv_freq = np.float32(1.0) / np.power(np.float32(ROPE_BASE), half)
    freqs = np.arange(S, dtype=np.float32)[:, None] * inv_freq[None, :]
    emb = np.concatenate([freqs, freqs], axis=1)  # [S, HD]
    cos = np.cos(emb).astype(np.float32)
    sin = np.sin(emb).astype(np.float32)
    sgn = np.where(np.arange(HD) < HD // 2, np.float32(-1.0), np.float32(1.0))
    sins = sin * sgn[None, :]  # signed sin for shift-based rotate_half

    latent_mean = x.mean(axis=1) @ np.asarray(w_kv_down, np.float32).T  # [B, R]
    v_mean = latent_mean @ np.asarray(w_kv_up, np.float32)[D:].T  # [B, D]
    fb_rows = v_mean @ np.asarray(w_out, np.float32).T  # [B, D]

    w_q = np.asarray(w_q, np.float32)
    w_kv_down = np.asarray(w_kv_down, np.float32)
    w_kv_up = np.asarray(w_kv_up, np.float32)
    w_out = np.asarray(w_out, np.float32)

    shared = {
        "wqT": _bf16(w_q.T),
        "wkvdT": _bf16(w_kv_down.T),
        "wkupT": _bf16(w_kv_up[:D].T),
        "wvupT": _bf16(w_kv_up[D:].T),  # [R, D], head h at cols h*64..
        "perm": _bf16(_perm()),
        "woutT": _bf16(w_out.T),
    }
    in_maps = []
    for c in range(8):
        b, sq = divmod(c, 4)
        s0 = sq * SQ
        m = dict(shared)
        m["xT"] = _bf16(x[b, s0 : s0 + SQ].T)
        m["xselT"] = _bf16(x[b, sel_pos[b]].T)
        m["cosq"] = _bf16(np.tile(cos[s0 : s0 + SQ].T, (2, 1)))
        m["sinq"] = _bf16(np.tile(sins[s0 : s0 + SQ].T, (2, 1)))
        m["cosk"] = _bf16(np.tile(cos[sel_pos[b]].T, (2, 1)))
        m["sink"] = _bf16(np.tile(sins[sel_pos[b]].T, (2, 1)))
        m["maskT"] = _bf16(
            sel_pos[b][:, None] <= (s0 + np.arange(SQ))[None, :]
        )
        in_maps.append(m)
    return in_maps, qmin, fb_rows


def build_nc():
    nc = bacc.Bacc("TRN2", target_bir_lowering=False)

    FD = BF16

    xT = nc.dram_tensor("xT", [D, SQ], FD, kind="ExternalInput")
    xselT = nc.dram_tensor("xselT", [D, KEYS], FD, kind="ExternalInput")
    wqT = nc.dram_tensor("wqT", [D, D], FD, kind="ExternalInput")
    wkvdT = nc.dram_tensor("wkvdT", [D, R], FD, kind="ExternalInput")
    wkupT = nc.dram_tensor("wkupT", [R, D], FD, kind="ExternalInput")
    wvupT = nc.dram_tensor("wvupT", [R, D], FD, kind="ExternalInput")
    perm = nc.dram_tensor("perm", [128, 128], FD, kind="ExternalInput")
    woutT = nc.dram_tensor("woutT", [D, D], FD, kind="ExternalInput")
    cosq = nc.dram_tensor("cosq", [128, SQ], FD, kind="ExternalInput")
    sinq = nc.dram_tensor("sinq", [128, SQ], FD, kind="ExternalInput")
    cosk = nc.dram_tensor("cosk", [128, KEYS], FD, kind="ExternalInput")
    sink = nc.dram_tensor("sink", [128, KEYS], FD, kind="ExternalInput")
    maskT = nc.dram_tensor("maskT", [KEYS, SQ], FD, kind="ExternalInput")
    out = nc.dram_tensor("out", [SQ, D], FD, kind="ExternalOutput")

    EXP = mybir.ActivationFunctionType.Exp

    with tile.TileContext(nc) as tc, ExitStack() as ctx:
        const = ctx.enter_context(tc.tile_pool(name="const", bufs=1))

        # warmup operand built on-device (no DMA dependency): the PE can
        # start spinning as soon as the engines finish instruction load
        warm_in = const.tile([128, 128], FD, tag="warm_in")
        nc.gpsimd.memset(warm_in[:], 0.125)

        # ---- persistent inputs, DMA'd in first-use order
        perm_sb = const.tile([128, 128], FD, tag="perm")
        nc.sync.dma_start(perm_sb[:], perm[:, :])
        xsel_sb = const.tile([128, DK, KEYS], FD, tag="xsel")
        nc.sync.dma_start(
            xsel_sb[:], xselT[:, :].rearrange("(k p) s -> p k s", p=128)
        )
        wkvd_sb = const.tile([128, DK, R], FD, tag="wkvd")
        nc.sync.dma_start(
            wkvd_sb[:], wkvdT[:, :].rearrange("(k p) r -> p k r", p=128)
        )
        wkup_sb = const.tile([128, D], FD, tag="wkup")
        nc.sync.dma_start(wkup_sb[:], wkupT[:, :])
        cosk_sb = const.tile([128, KEYS], FD, tag="cosk")
        nc.sync.dma_start(cosk_sb[:], cosk[:, :])
        sink_sb = const.tile([128, KEYS], FD, tag="sink")
        nc.sync.dma_start(sink_sb[:], sink[:, :])
        wvup_sb = const.tile([128, D], FD, tag="wvup")
        nc.sync.dma_start(wvup_sb[:], wvupT[:, :])
        xT_sb = const.tile([128, DK, SQ], FD, tag="xT")
        nc.sync.dma_start(
            xT_sb[:], xT[:, :].rearrange("(k p) s -> p k s", p=128)
        )
        cosq_sb = const.tile([128, SQ], FD, tag="cosq")
        nc.sync.dma_start(cosq_sb[:], cosq[:, :])
        sinq_sb = const.tile([128, SQ], FD, tag="sinq")
        nc.sync.dma_start(sinq_sb[:], sinq[:, :])
        mask_sb = const.tile([128, 2, SQ], FD, tag="mask")
        nc.sync.dma_start(
            mask_sb[:], maskT[:, :].rearrange("(m p) s -> p m s", p=128)
        )
        # wq sliced by output (head-pair) chunk so q-proj of pair p can
        # start as soon as its slice lands
        wq_sb = const.tile([128, DK, D], FD, tag="wq")
        for p2 in range(0, CK, 2):
            nc.sync.dma_start(
                wq_sb[:, :, p2 * 128 : (p2 + 2) * 128],
                wqT[:, p2 * 128 : (p2 + 2) * 128].rearrange(
                    "(k p) c -> p k c", p=128
                ),
            )
        ones64_sb = const.tile([128, 64], FD, tag="ones64")
        wout_sb = const.tile([128, DK, D], FD, tag="wout")
        nc.sync.dma_start(
            wout_sb[:], woutT[:, :].rearrange("(k p) c -> p k c", p=128)
        )

        # ---- results that span stages
        kT_sb = const.tile([128, CK, KEYS], FD, tag="kT")
        v_sb = const.tile([128, 2, D], FD, tag="v")
        yT_sb = const.tile([128, CK, SQ], FD, tag="yT")

        # ================= warmup: keep PE busy while DMAs land ==========
        with tc.tile_pool(name="ps_warm", bufs=1, space="PSUM") as ps_warm:
            warm_ps = ps_warm.tile([128, 128], F32, tag="warm")
            for _ in range(N_WARMUP):
                nc.tensor.matmul(
                    warm_ps[:], warm_in[:], warm_in[:], start=True, stop=True
                )
            nc.gpsimd.memset(ones64_sb[:], 1.0)

        # ================= stages A-C ====================================
        with (
            tc.tile_pool(name="work", bufs=3) as work,
            tc.tile_pool(name="ps_e", bufs=3, space="PSUM") as ps_e,
            tc.tile_pool(name="ps_r", bufs=2, space="PSUM") as ps_r,
        ):
            # ---- stage A: latentT at selected positions [R, KEYS]
            lat_ps = ps_e.tile([128, KEYS], F32, tag="early")
            for dk in range(DK):
                nc.tensor.matmul(
                    lat_ps[:],
                    wkvd_sb[:, dk, :],
                    xsel_sb[:, dk, :],
                    start=(dk == 0),
                    stop=(dk == DK - 1),
                )
            lat_sb = const.tile([128, KEYS], FD, tag="lat")
            nc.scalar.copy(lat_sb[:], lat_ps[:])

            # ---- stages B+C interleaved: kT chunks + RoPE, v chunks.
            # C matmuls slot into the gaps while B waits on PSUM copies.
            cjobs = [(mk, nh) for mk in range(2) for nh in range(2)]
            for ck in range(CK):
                k_ps = ps_e.tile([128, KEYS], F32, tag="early")
                nc.tensor.matmul(
                    k_ps[:],
                    wkup_sb[:, ck * 128 : (ck + 1) * 128],
                    lat_sb[:],
                    start=True,
                    stop=True,
                )
                k_raw = work.tile([128, KEYS], FD, tag="k_raw")
                if ck % 2 == 0:
                    nc.scalar.copy(k_raw[:], k_ps[:])
                else:
                    nc.vector.tensor_copy(k_raw[:], k_ps[:])
                k_rot = ps_r.tile([128, KEYS], F32, tag="rot")
                nc.tensor.matmul(
                    k_rot[:], perm_sb[:], k_raw[:], start=True, stop=True
                )
                # all-vector RoPE here: gpsimd TT is ~2x slower and was
                # serializing kT production, gating the whole fused stage
                kt1 = work.tile([128, KEYS], FD, tag="kt1")
                nc.vector.tensor_mul(kt1[:], k_raw[:], cosk_sb[:])
                kt2 = work.tile([128, KEYS], FD, tag="kt2")
                nc.vector.tensor_mul(kt2[:], k_rot[:], sink_sb[:])
                nc.vector.tensor_add(kT_sb[:, ck, :], kt1[:], kt2[:])
                if ck % 2 == 1 and cjobs:
                    mk, nh = cjobs.pop(0)
                    v_ps = ps_e.tile([128, 512], F32, tag="earlyv")
                    nc.tensor.matmul(
                        v_ps[:],
                        lat_sb[:, mk * 128 : (mk + 1) * 128],
                        wvup_sb[:, nh * 512 : (nh + 1) * 512],
                        start=True,
                        stop=True,
                    )
                    if nh % 2 == 0:
                        nc.scalar.copy(v_sb[:, mk, nh * 512 : (nh + 1) * 512], v_ps[:])
                    else:
                        nc.vector.tensor_copy(
                            v_sb[:, mk, nh * 512 : (nh + 1) * 512], v_ps[:]
                        )

        # ================= fused stage D+E per head pair ==================
        # PSUM plan (8 banks): q 1 | rot/z/outT2 ring 3 | sc 4.
        # Scores issue mk-major so hi=0/hi=1 land in different PE row
        # groups back-to-back (concurrent); Z/PV alternate hi so the 64-col
        # tiles pair up in opposite column groups. sc layout: [hi*2+mk].
        with (
            tc.tile_pool(name="epool", bufs=3) as epool,
            tc.tile_pool(name="work2", bufs=3) as work2,
            tc.tile_pool(name="ps_q", bufs=1, space="PSUM") as ps_q,
            tc.tile_pool(name="ps_mix", bufs=1, space="PSUM") as ps_mix,
            tc.tile_pool(name="ps_sc", bufs=1, space="PSUM") as ps_sc,
        ):
            # Software-pipelined: q-proj + RoPE of pair p+1 issue BEFORE
            # the attention of pair p, so each engine's strict FIFO has
            # p+1's early ops ahead of p's late ops (otherwise the loop
            # period degenerates to the full serial chain latency).
            qTr_tiles = [None] * CK

            def rope(p):
                q_ps = ps_q.tile([128, SQ], F32, tag="qT", name="q_ps")
                for dk in range(DK):
                    nc.tensor.matmul(
                        q_ps[:],
                        wq_sb[:, dk, p * 128 : (p + 1) * 128],
                        xT_sb[:, dk, :],
                        start=(dk == 0),
                        stop=(dk == DK - 1),
                    )
                q_raw = work2.tile([128, SQ], FD, tag="q_raw", name="q_raw")
                nc.scalar.copy(q_raw[:], q_ps[:])
                q_rot = ps_mix.tile([128, SQ], F32, tag="rot", name="q_rot")
                nc.tensor.matmul(
                    q_rot[:], perm_sb[:], q_raw[:], start=True, stop=True
                )
                qt1 = work2.tile([128, SQ], FD, tag="qt1", name="qt1")
                nc.gpsimd.tensor_mul(qt1[:], q_raw[:], cosq_sb[:])
                qt2 = work2.tile([128, SQ], FD, tag="qt2", name="qt2")
                nc.vector.tensor_mul(qt2[:], q_rot[:], sinq_sb[:])
                qTr = work2.tile([128, SQ], FD, tag="qTr", name="qTr", bufs=2)
                nc.gpsimd.tensor_add(qTr[:], qt1[:], qt2[:])
                qTr_tiles[p] = qTr

            rope(0)
            for p in range(CK):
                if p + 1 < CK:
                    rope(p + 1)
                qTr = qTr_tiles[p]

                # ---- attention for this head pair
                z_ps = ps_mix.tile([128, SQ], F32, tag="z")
                outT2 = ps_mix.tile([128, SQ], F32, tag="o")
                sc = [
                    ps_sc.tile([128, 2, SQ], F32, tag="sca", name="sca"),
                    ps_sc.tile([128, 2, SQ], F32, tag="scb", name="scb"),
                ]
                for mk in range(2):
                    for hi in range(2):
                        pb = hi * 64
                        nc.tensor.matmul(
                            sc[hi][:, mk, :],
                            kT_sb[pb : pb + 64, p, mk * 128 : (mk + 1) * 128],
                            qTr[pb : pb + 64, :],
                            start=True,
                            stop=True,
                        )
                expT = [
                    epool.tile([128, 2, SQ], FD, tag="expTa", name="expTa"),
                    epool.tile([128, 2, SQ], FD, tag="expTb", name="expTb"),
                ]
                expM = [
                    epool.tile([128, 2, SQ], FD, tag="expMa", name="expMa"),
                    epool.tile([128, 2, SQ], FD, tag="expMb", name="expMb"),
                ]
                for hi in range(2):
                    nc.scalar.activation(
                        expT[hi][:].rearrange("p m s -> p (m s)"),
                        sc[hi][:].rearrange("p m s -> p (m s)"),
                        EXP,
                        scale=0.125,
                    )
                    # both masks on vector: clean single-engine tiles keep
                    # the DVE in its packed bf16 mode and off gpsimd's port
                    nc.vector.tensor_mul(
                        expM[hi][:].rearrange("p m s -> p (m s)"),
                        expT[hi][:].rearrange("p m s -> p (m s)"),
                        mask_sb[:].rearrange("p m s -> p (m s)"),
                    )
                for mk in range(2):
                    for hi in range(2):
                        pb = hi * 64
                        nc.tensor.matmul(
                            z_ps[pb : pb + 64, :],
                            ones64_sb[:],
                            expM[hi][:, mk, :],
                            start=(mk == 0),
                            stop=(mk == 1),
                        )
                for mk in range(2):
                    for hi in range(2):
                        pb = hi * 64
                        h = 2 * p + hi
                        nc.tensor.matmul(
                            outT2[pb : pb + 64, :],
                            v_sb[:, mk, h * 64 : (h + 1) * 64],
                            expM[hi][:, mk, :],
                            start=(mk == 0),
                            stop=(mk == 1),
                        )
                zr = work2.tile([128, SQ], F32, tag="zr")
                nc.vector.reciprocal_approx_fast(zr[:], z_ps[:])
                nc.vector.tensor_mul(yT_sb[:, p, :], outT2[:], zr[:])

            # filler matmuls: bridge the PE-idle window between the last
            # pair's softmax tail and stage F so the HAM clock stays warm
            warm2 = ps_q.tile([128, SQ], F32, tag="qT", name="warm2")
            for _ in range(10):
                nc.tensor.matmul(
                    warm2[:, 0:128], warm_in[:], warm_in[:], start=True, stop=True
                )

        # ================= stage F: out = yT.T @ woutT ====================
        with (
            tc.tile_pool(name="ps_w", bufs=2, space="PSUM") as ps_w,
            tc.tile_pool(name="ost", bufs=2) as ost,
        ):
            for st in range(4):
                outp = ps_w.tile([128, 2, 512], F32, tag="wps")
                for ck in range(CK):
                    for dh in range(2):
                        nc.tensor.matmul(
                            outp[:, dh, :],
                            yT_sb[:, ck, st * 128 : (st + 1) * 128],
                            wout_sb[:, ck, dh * 512 : (dh + 1) * 512],
                            start=(ck == 0),
                            stop=(ck == CK - 1),
                        )
                o_sb = ost.tile([128, D], FD, tag="osb")
                nc.scalar.copy(o_sb[:, 0:512], outp[:, 0, :])
                nc.sync.dma_start(out[st * 128 : (st + 1) * 128, 0:512], o_sb[:, 0:512])
                nc.vector.tensor_copy(o_sb[:, 512:1024], outp[:, 1, :])
                nc.sync.dma_start(
                    out[st * 128 : (st + 1) * 128, 512:1024], o_sb[:, 512:1024]
                )

    nc.compile()
    return nc


_NC_CACHE = {}


def _get_nc():
    if "nc" not in _NC_CACHE:
        _NC_CACHE["nc"] = build_nc()
    return _NC_CACHE["nc"]


TRACE = False  # set by test harness to capture an NTFF profile
LAST_RESULTS = None


def kernel(x, w_q, w_kv_down, w_kv_up, w_out, w_scorer):
    global LAST_RESULTS
    from concourse.bass_utils import run_bass_kernel_spmd

    in_maps, qmin, fb_rows = host_prep(x, w_q, w_kv_down, w_kv_up, w_out, w_scorer)
    nc = _get_nc()
    res = run_bass_kernel_spmd(nc, in_maps, core_ids=list(range(8)), trace=TRACE)
    LAST_RESULTS = res
    out = np.empty((B, S, D), np.float32)
    for c in range(8):
        b, sq = divmod(c, 4)
        out[b, sq * SQ : (sq + 1) * SQ] = np.asarray(
            res.results[c]["out"], dtype=np.float32
        )
    for b in range(B):
        if qmin[b] > 0:
            out[b, : qmin[b]] = fb_rows[b]
    return out


# revision 24
# speedup vs baseline: 1.2119x; 1.2119x over previous
"""BlockSparseMLA Trainium2 kernel (v2).

Sharding: 8 cores = 2 batches x 4 seq-quarters. Each core computes all 16
heads for its 512 queries. Host does block scoring / top-k, gathers
selected positions, builds the causal mask over selected keys, and patches
degenerate all-masked rows with a host-computed rank-1 fallback.

Device layouts are all "transposed" (feature dim on partitions):
  qT [c=16h*64, s]  kT [c, keys]  v [keys, c]  scoresT/expT [keys, s]
  yT [c, s]  out [s, dout]
All matmul operands are bf16 (fp32r measured ~1.8 cyc/row vs bf16 1.0);
PSUM accumulates fp32.

v2 structure:
- ~32 warmup matmuls on the perm tile while DMA streams in, so the PE HAM
  clock-gate reaches 8/8 before real work starts and stays there.
- Stages: A latent -> B kT+RoPE -> C v -> fused {q-proj chunk p, RoPE,
  scores, exp, mask, Z, PV, 1/Z, yT} per head-pair p -> F out-proj.
  Fusing D+E gives the PE queue q-proj work of pair p+1 to chew on while
  the scalar/vector/gpsimd engines run pair p's softmax chain.
- Scores use 64-partition row tiles (hi=0/1 in different row groups, can
  overlap); Z/PV are 64-column col tiles (hi halves of the PSUM tile) with
  an unpadded v layout.
- 1/Z via reciprocal_approx_fast (z=0 rows give garbage confined to dead
  queries, overwritten on host).
- Stage F runs st-outer so each 128-query output strip is evacuated and
  DMA'd while the next accumulates; out is bf16, host casts to fp32.
"""

import sys

import numpy as np

sys.path.insert(0, "/opt/trn_rl_repo")

from contextlib import ExitStack

import ml_dtypes

import concourse.bacc as bacc
import concourse.bass as bass
import concourse.mybir as mybir
import concourse.tile as tile

B, S, D = 2, 2048, 1024
H, HD, R = 16, 64, 128
BLOCK, TOPK = 64, 4
ROPE_BASE = 100000.0
SQ = S // 4
KEYS = TOPK * BLOCK  # 256
CK = D // 128  # c chunks (2 heads each)
DK = D // 128  # d chunks
F32 = mybir.dt.float32
BF16 = mybir.dt.bfloat16
NPBF16 = ml_dtypes.bfloat16
N_WARMUP = 48  # ~5us of N=128 matmuls at the cold 1.2 GHz clock


def _bf16(a):
    return np.ascontiguousarray(np.asarray(a, dtype=np.float32).astype(NPBF16))


def _perm():
    """[128, 128] block-diag rotate-half permutation: out[p] = in[p^32
    within each 64-block] (symmetric)."""
    P = np.zeros((128, 128), np.float32)
    for pp in range(128):
        blk, e = divmod(pp, 64)
        s = blk * 64 + (e + 32 if e < 32 else e - 32)
        P[s, pp] = 1.0
    return P


def host_prep(x, w_q, w_kv_down, w_kv_up, w_out, w_scorer):
    """Returns (in_maps for 8 cores, qmin[B], fb_rows[B, D])."""
    x = np.asarray(x, dtype=np.float32)
    nb = S // BLOCK

    reps = x.reshape(B, nb, BLOCK, D).mean(axis=2)
    scores = reps @ np.asarray(w_scorer, np.float32)[0]
    top = np.argsort(-scores, axis=1, kind="stable")[:, :TOPK]
    sel_blocks = np.sort(top, axis=1)
    qmin = sel_blocks[:, 0] * BLOCK
    sel_pos = (
        sel_blocks[:, :, None] * BLOCK + np.arange(BLOCK)[None, None, :]
    ).reshape(B, KEYS)

    half = np.arange(0, HD, 2, dtype=np.float32) / np.float32(HD)
    inv_freq = np.float32(1.0) / np.power(np.float32(ROPE_BASE), half)
    freqs = np.arange(S, dtype=np.float32)[:, None] * inv_freq[None, :]
    emb = np.concatenate([freqs, freqs], axis=1)  # [S, HD]
    cos = np.cos(emb).astype(np.float32)
    sin = np.sin(emb).astype(np.float32)
    sgn = np.where(np.arange(HD) < HD // 2, np.float32(-1.0), np.float32(1.0))
    sins = sin * sgn[None, :]  # signed sin for shift-based rotate_half

    latent_mean = x.mean(axis=1) @ np.asarray(w_kv_down, np.float32).T  # [B, R]
    v_mean = latent_mean @ np.asarray(w_kv_up, np.float32)[D:].T  # [B, D]
    fb_rows = v_mean @ np.asarray(w_out, np.float32).T  # [B, D]

    w_q = np.asarray(w_q, np.float32)
    w_kv_down = np.asarray(w_kv_down, np.float32)
    w_kv_up = np.asarray(w_kv_up, np.float32)
    w_out = np.asarray(w_out, np.float32)

    # rotate-half as an index permutation of the head dim (within each
    # 64-block swap the 32-halves); applied to w_kv_up columns host-side so
    # the k-rotation is a direct matmul on latent (no PE-queue dependency
    # on the k evacuation)
    rot_idx = np.arange(D).reshape(-1, 64)
    rot_idx = np.concatenate([rot_idx[:, 32:], rot_idx[:, :32]], axis=1).reshape(-1)
    wkupT = _bf16(w_kv_up[:D].T)

    shared = {
        "wqT": _bf16(w_q.T),
        "wkvdT": _bf16(w_kv_down.T),
        "wkupT": wkupT,
        "wkuprT": np.ascontiguousarray(wkupT[:, rot_idx]),
        "wvupT": _bf16(w_kv_up[D:].T),  # [R, D], head h at cols h*64..
        "perm": _bf16(_perm()),
        "woutT": _bf16(w_out.T),
    }
    in_maps = []
    for c in range(8):
        b, sq = divmod(c, 4)
        s0 = sq * SQ
        m = dict(shared)
        m["xT"] = _bf16(x[b, s0 : s0 + SQ].T)
        m["xselT"] = _bf16(x[b, sel_pos[b]].T)
        m["cosq"] = _bf16(np.tile(cos[s0 : s0 + SQ].T, (2, 1)))
        m["sinq"] = _bf16(np.tile(sins[s0 : s0 + SQ].T, (2, 1)))
        m["cosk"] = _bf16(np.tile(cos[sel_pos[b]].T, (2, 1)))
        m["sink"] = _bf16(np.tile(sins[sel_pos[b]].T, (2, 1)))
        m["maskT"] = _bf16(
            sel_pos[b][:, None] <= (s0 + np.arange(SQ))[None, :]
        )
        in_maps.append(m)
    return in_maps, qmin, fb_rows


def build_nc():
    nc = bacc.Bacc("TRN2", target_bir_lowering=False)

    FD = BF16

    xT = nc.dram_tensor("xT", [D, SQ], FD, kind="ExternalInput")
    xselT = nc.dram_tensor("xselT", [D, KEYS], FD, kind="ExternalInput")
    wqT = nc.dram_tensor("wqT", [D, D], FD, kind="ExternalInput")
    wkvdT = nc.dram_tensor("wkvdT", [D, R], FD, kind="ExternalInput")
    wkupT = nc.dram_tensor("wkupT", [R, D], FD, kind="ExternalInput")
    wkuprT = nc.dram_tensor("wkuprT", [R, D], FD, kind="ExternalInput")
    wvupT = nc.dram_tensor("wvupT", [R, D], FD, kind="ExternalInput")
    perm = nc.dram_tensor("perm", [128, 128], FD, kind="ExternalInput")
    woutT = nc.dram_tensor("woutT", [D, D], FD, kind="ExternalInput")
    cosq = nc.dram_tensor("cosq", [128, SQ], FD, kind="ExternalInput")
    sinq = nc.dram_tensor("sinq", [128, SQ], FD, kind="ExternalInput")
    cosk = nc.dram_tensor("cosk", [128, KEYS], FD, kind="ExternalInput")
    sink = nc.dram_tensor("sink", [128, KEYS], FD, kind="ExternalInput")
    maskT = nc.dram_tensor("maskT", [KEYS, SQ], FD, kind="ExternalInput")
    out = nc.dram_tensor("out", [SQ, D], FD, kind="ExternalOutput")

    EXP = mybir.ActivationFunctionType.Exp

    with tile.TileContext(nc) as tc, ExitStack() as ctx:
        const = ctx.enter_context(tc.tile_pool(name="const", bufs=1))

        # warmup operand built on-device (no DMA dependency): the PE can
        # start spinning as soon as the engines finish instruction load
        warm_in = const.tile([128, 128], FD, tag="warm_in")
        nc.gpsimd.memset(warm_in[:], 0.125)

        # ---- persistent inputs, DMA'd in first-use order
        perm_sb = const.tile([128, 128], FD, tag="perm")
        nc.sync.dma_start(perm_sb[:], perm[:, :])
        xsel_sb = const.tile([128, DK, KEYS], FD, tag="xsel")
        nc.sync.dma_start(
            xsel_sb[:], xselT[:, :].rearrange("(k p) s -> p k s", p=128)
        )
        wkvd_sb = const.tile([128, DK, R], FD, tag="wkvd")
        nc.sync.dma_start(
            wkvd_sb[:], wkvdT[:, :].rearrange("(k p) r -> p k r", p=128)
        )
        wkup_sb = const.tile([128, D], FD, tag="wkup")
        nc.sync.dma_start(wkup_sb[:], wkupT[:, :])
        wkupr_sb = const.tile([128, D], FD, tag="wkupr")
        nc.sync.dma_start(wkupr_sb[:], wkuprT[:, :])
        cosk_sb = const.tile([128, KEYS], FD, tag="cosk")
        nc.sync.dma_start(cosk_sb[:], cosk[:, :])
        sink_sb = const.tile([128, KEYS], FD, tag="sink")
        nc.sync.dma_start(sink_sb[:], sink[:, :])
        wvup_sb = const.tile([128, D], FD, tag="wvup")
        nc.sync.dma_start(wvup_sb[:], wvupT[:, :])
        xT_sb = const.tile([128, DK, SQ], FD, tag="xT")
        nc.sync.dma_start(
            xT_sb[:], xT[:, :].rearrange("(k p) s -> p k s", p=128)
        )
        cosq_sb = const.tile([128, SQ], FD, tag="cosq")
        nc.sync.dma_start(cosq_sb[:], cosq[:, :])
        sinq_sb = const.tile([128, SQ], FD, tag="sinq")
        nc.sync.dma_start(sinq_sb[:], sinq[:, :])
        mask_sb = const.tile([128, 2, SQ], FD, tag="mask")
        nc.sync.dma_start(
            mask_sb[:], maskT[:, :].rearrange("(m p) s -> p m s", p=128)
        )
        # wq sliced by output (head-pair) chunk so q-proj of pair p can
        # start as soon as its slice lands
        wq_sb = const.tile([128, DK, D], FD, tag="wq")
        for p2 in range(0, CK, 2):
            nc.sync.dma_start(
                wq_sb[:, :, p2 * 128 : (p2 + 2) * 128],
                wqT[:, p2 * 128 : (p2 + 2) * 128].rearrange(
                    "(k p) c -> p k c", p=128
                ),
            )
        ones64_sb = const.tile([128, 64], FD, tag="ones64")
        wout_sb = const.tile([128, DK, D], FD, tag="wout")
        nc.sync.dma_start(
            wout_sb[:], woutT[:, :].rearrange("(k p) c -> p k c", p=128)
        )

        # ---- results that span stages
        kT_sb = const.tile([128, CK, KEYS], FD, tag="kT")
        v_sb = const.tile([128, 2, D], FD, tag="v")
        yT_sb = const.tile([128, CK, SQ], FD, tag="yT")

        # ================= warmup: keep PE busy while DMAs land ==========
        with tc.tile_pool(name="ps_warm", bufs=1, space="PSUM") as ps_warm:
            warm_ps = ps_warm.tile([128, 128], F32, tag="warm")
            for _ in range(N_WARMUP):
                nc.tensor.matmul(
                    warm_ps[:], warm_in[:], warm_in[:], start=True, stop=True
                )
            nc.gpsimd.memset(ones64_sb[:], 1.0)

        # ================= stages A-C ====================================
        with (
            tc.tile_pool(name="work", bufs=3) as work,
            tc.tile_pool(name="ps_e", bufs=3, space="PSUM") as ps_e,
            tc.tile_pool(name="ps_r", bufs=2, space="PSUM") as ps_r,
        ):
            # ---- stage A: latentT at selected positions [R, KEYS]
            lat_ps = ps_e.tile([128, KEYS], F32, tag="early")
            for dk in range(DK):
                nc.tensor.matmul(
                    lat_ps[:],
                    wkvd_sb[:, dk, :],
                    xsel_sb[:, dk, :],
                    start=(dk == 0),
                    stop=(dk == DK - 1),
                )
            lat_sb = const.tile([128, KEYS], FD, tag="lat")
            nc.scalar.copy(lat_sb[:], lat_ps[:])

            # ---- stages B+C interleaved: kT chunks + RoPE, v chunks.
            # k_rot comes from the host-permuted weights (direct matmul on
            # latent) so no PE instruction ever waits on a PSUM evacuation.
            cjobs = [(mk, nh) for mk in range(2) for nh in range(2)]
            for ck in range(CK):
                k_ps = ps_e.tile([128, KEYS], F32, tag="early")
                nc.tensor.matmul(
                    k_ps[:],
                    wkup_sb[:, ck * 128 : (ck + 1) * 128],
                    lat_sb[:],
                    start=True,
                    stop=True,
                )
                kr_ps = ps_r.tile([128, KEYS], F32, tag="rot")
                nc.tensor.matmul(
                    kr_ps[:],
                    wkupr_sb[:, ck * 128 : (ck + 1) * 128],
                    lat_sb[:],
                    start=True,
                    stop=True,
                )
                k_raw = work.tile([128, KEYS], FD, tag="k_raw")
                if ck % 2 == 0:
                    nc.scalar.copy(k_raw[:], k_ps[:])
                else:
                    nc.vector.tensor_copy(k_raw[:], k_ps[:])
                kt1 = work.tile([128, KEYS], FD, tag="kt1")
                nc.gpsimd.tensor_mul(kt1[:], k_raw[:], cosk_sb[:])
                kt2 = work.tile([128, KEYS], FD, tag="kt2")
                nc.vector.tensor_mul(kt2[:], kr_ps[:], sink_sb[:])
                nc.gpsimd.tensor_add(kT_sb[:, ck, :], kt1[:], kt2[:])
                if ck % 2 == 1 and cjobs:
                    mk, nh = cjobs.pop(0)
                    v_ps = ps_e.tile([128, 512], F32, tag="earlyv")
                    nc.tensor.matmul(
                        v_ps[:],
                        lat_sb[:, mk * 128 : (mk + 1) * 128],
                        wvup_sb[:, nh * 512 : (nh + 1) * 512],
                        start=True,
                        stop=True,
                    )
                    if nh % 2 == 0:
                        nc.scalar.copy(v_sb[:, mk, nh * 512 : (nh + 1) * 512], v_ps[:])
                    else:
                        nc.vector.tensor_copy(
                            v_sb[:, mk, nh * 512 : (nh + 1) * 512], v_ps[:]
                        )

        # ================= fused stage D+E per head pair ==================
        # PSUM plan (8 banks): q 1 | rot/z/outT2 ring 3 | sc 4.
        # Scores issue mk-major so hi=0/hi=1 land in different PE row
        # groups back-to-back (concurrent); Z/PV alternate hi so the 64-col
        # tiles pair up in opposite column groups. sc layout: [hi*2+mk].
        with (
            tc.tile_pool(name="epool", bufs=3) as epool,
            tc.tile_pool(name="work2", bufs=3) as work2,
            tc.tile_pool(name="ps_q", bufs=1, space="PSUM") as ps_q,
            tc.tile_pool(name="ps_mix", bufs=1, space="PSUM") as ps_mix,
            tc.tile_pool(name="ps_sc", bufs=1, space="PSUM") as ps_sc,
        ):
            # Software-pipelined: q-proj + RoPE of pair p+1 issue around
            # the attention of pair p, so each engine's strict FIFO has
            # p+1's early ops ahead of p's late ops. The perm matmul of
            # p+1 is emitted AFTER scores(p) so the PE queue never stalls
            # waiting for the scalar PSUM evacuation it depends on.
            qTr_tiles = [None] * CK
            rope_state = {}

            def rope_pre(p):
                q_ps = ps_q.tile([128, SQ], F32, tag="qT", name="q_ps")
                for dk in range(DK):
                    nc.tensor.matmul(
                        q_ps[:],
                        wq_sb[:, dk, p * 128 : (p + 1) * 128],
                        xT_sb[:, dk, :],
                        start=(dk == 0),
                        stop=(dk == DK - 1),
                    )
                q_raw = work2.tile([128, SQ], FD, tag="q_raw", name="q_raw")
                nc.scalar.copy(q_raw[:], q_ps[:])
                qt1 = work2.tile([128, SQ], FD, tag="qt1", name="qt1")
                nc.gpsimd.tensor_mul(qt1[:], q_raw[:], cosq_sb[:])
                rope_state[p] = (q_raw, qt1)

            def rope_post(p):
                q_raw, qt1 = rope_state.pop(p)
                q_rot = ps_mix.tile([128, SQ], F32, tag="rot", name="q_rot")
                nc.tensor.matmul(
                    q_rot[:], perm_sb[:], q_raw[:], start=True, stop=True
                )
                qt2 = work2.tile([128, SQ], FD, tag="qt2", name="qt2")
                nc.vector.tensor_mul(qt2[:], q_rot[:], sinq_sb[:])
                qTr = work2.tile([128, SQ], FD, tag="qTr", name="qTr", bufs=2)
                nc.gpsimd.tensor_add(qTr[:], qt1[:], qt2[:])
                qTr_tiles[p] = qTr

            rope_pre(0)
            rope_post(0)
            for p in range(CK):
                if p + 1 < CK:
                    rope_pre(p + 1)
                qTr = qTr_tiles[p]

                # ---- attention for this head pair
                z_ps = ps_mix.tile([128, SQ], F32, tag="z")
                outT2 = ps_mix.tile([128, SQ], F32, tag="o")
                sc = [
                    ps_sc.tile([128, 2, SQ], F32, tag="sca", name="sca"),
                    ps_sc.tile([128, 2, SQ], F32, tag="scb", name="scb"),
                ]
                for mk in range(2):
                    for hi in range(2):
                        pb = hi * 64
                        nc.tensor.matmul(
                            sc[hi][:, mk, :],
                            kT_sb[pb : pb + 64, p, mk * 128 : (mk + 1) * 128],
                            qTr[pb : pb + 64, :],
                            start=True,
                            stop=True,
                        )
                if p + 1 < CK:
                    rope_post(p + 1)
                expT = [
                    epool.tile([128, 2, SQ], FD, tag="expTa", name="expTa"),
                    epool.tile([128, 2, SQ], FD, tag="expTb", name="expTb"),
                ]
                expM = [
                    epool.tile([128, 2, SQ], FD, tag="expMa", name="expMa"),
                    epool.tile([128, 2, SQ], FD, tag="expMb", name="expMb"),
                ]
                for hi in range(2):
                    nc.scalar.activation(
                        expT[hi][:].rearrange("p m s -> p (m s)"),
                        sc[hi][:].rearrange("p m s -> p (m s)"),
                        EXP,
                        scale=0.125,
                    )
                    # both masks on vector: clean single-engine tiles keep
                    # the DVE in its packed bf16 mode and off gpsimd's port
                    nc.vector.tensor_mul(
                        expM[hi][:].rearrange("p m s -> p (m s)"),
                        expT[hi][:].rearrange("p m s -> p (m s)"),
                        mask_sb[:].rearrange("p m s -> p (m s)"),
                    )
                for mk in range(2):
                    for hi in range(2):
                        pb = hi * 64
                        nc.tensor.matmul(
                            z_ps[pb : pb + 64, :],
                            ones64_sb[:],
                            expM[hi][:, mk, :],
                            start=(mk == 0),
                            stop=(mk == 1),
                        )
                for mk in range(2):
                    for hi in range(2):
                        pb = hi * 64
                        h = 2 * p + hi
                        nc.tensor.matmul(
                            outT2[pb : pb + 64, :],
                            v_sb[:, mk, h * 64 : (h + 1) * 64],
                            expM[hi][:, mk, :],
                            start=(mk == 0),
                            stop=(mk == 1),
                        )
                zr = work2.tile([128, SQ], F32, tag="zr")
                nc.vector.reciprocal_approx_fast(zr[:], z_ps[:])
                nc.vector.tensor_mul(yT_sb[:, p, :], outT2[:], zr[:])

            # filler matmuls: bridge the PE-idle window between the last
            # pair's softmax tail and stage F so the HAM clock stays warm
            warm2 = ps_q.tile([128, SQ], F32, tag="qT", name="warm2")
            for _ in range(10):
                nc.tensor.matmul(
                    warm2[:, 0:128], warm_in[:], warm_in[:], start=True, stop=True
                )

        # ================= stage F: out = yT.T @ woutT ====================
        with (
            tc.tile_pool(name="ps_w", bufs=2, space="PSUM") as ps_w,
            tc.tile_pool(name="ost", bufs=2) as ost,
        ):
            for st in range(4):
                outp = ps_w.tile([128, 2, 512], F32, tag="wps")
                for ck in range(CK):
                    for dh in range(2):
                        nc.tensor.matmul(
                            outp[:, dh, :],
                            yT_sb[:, ck, st * 128 : (st + 1) * 128],
                            wout_sb[:, ck, dh * 512 : (dh + 1) * 512],
                            start=(ck == 0),
                            stop=(ck == CK - 1),
                        )
                o_sb = ost.tile([128, D], FD, tag="osb")
                nc.scalar.copy(o_sb[:, 0:512], outp[:, 0, :])
                nc.sync.dma_start(out[st * 128 : (st + 1) * 128, 0:512], o_sb[:, 0:512])
                nc.vector.tensor_copy(o_sb[:, 512:1024], outp[:, 1, :])
                nc.sync.dma_start(
                    out[st * 128 : (st + 1) * 128, 512:1024], o_sb[:, 512:1024]
                )

    nc.compile()
    return nc


_NC_CACHE = {}


def _get_nc():
    if "nc" not in _NC_CACHE:
        _NC_CACHE["nc"] = build_nc()
    return _NC_CACHE["nc"]


TRACE = False  # set by test harness to capture an NTFF profile
LAST_RESULTS = None


def kernel(x, w_q, w_kv_down, w_kv_up, w_out, w_scorer):
    global LAST_RESULTS
    from concourse.bass_utils import run_bass_kernel_spmd

    in_maps, qmin, fb_rows = host_prep(x, w_q, w_kv_down, w_kv_up, w_out, w_scorer)
    nc = _get_nc()
    res = run_bass_kernel_spmd(nc, in_maps, core_ids=list(range(8)), trace=TRACE)
    LAST_RESULTS = res
    out = np.empty((B, S, D), np.float32)
    for c in range(8):
        b, sq = divmod(c, 4)
        out[b, sq * SQ : (sq + 1) * SQ] = np.asarray(
            res.results[c]["out"], dtype=np.float32
        )
    for b in range(B):
        if qmin[b] > 0:
            out[b, : qmin[b]] = fb_rows[b]
    return out
